# revision 1
# baseline (speedup 1.0000x reference)
"""GCN (4x GCNConv + global mean pool + MLP head) on 8 Trainium2 NeuronCores.

Strategy (node sharding, per the usual GNN partitioning):
  - Host: relabel the 100k nodes into 8 cores x 98 windows x 128 slots via a
    capacity-constrained bin packing so that every window of 128 nodes holds
    at most SUB*128 incident edge slots (in-edges + one self-loop per node).
  - Per layer l: each core computes h = x_shard @ W_l (PE), AllGathers h so
    every core holds the full [100352, d] matrix, then aggregates its own
    windows: one indirect-DMA gather of SUB*128 rows per window, a per-edge
    scale+select matrix S built on DVE (iota == dst_local) * norm, and
    PE matmuls S^T @ G accumulated in PSUM. Self-loop term is folded in as an
    extra edge with weight dinv^2. ReLU on ACT writes the bf16 output shard.
  - Mean pool: PE matmuls P_w^T @ h4_w accumulated over windows ([64 graphs x
    64 feats]), AllReduce over cores, scale by 1/count, then the tiny MLP head
    (computed redundantly on every core; host takes core 0).

All heavy traffic (h exchange, gathers, matmuls) runs in bf16 with f32
accumulation; biases in this model are zero but are still applied if nonzero.
"""

import heapq
import math

import numpy as np
import ml_dtypes

# ---------------------------------------------------------------- constants
N_NODES = 100000
N_EDGES = 400000
N_GRAPHS = 64
DIMS = [(512, 512), (512, 256), (256, 128), (128, 64)]
N_CORES = 8
P = 128          # partitions / slots per window
W_WINDOWS = 98   # windows per core -> 12544 slots/core
SLOTS = W_WINDOWS * P          # 12544
SLOTS_ALL = SLOTS * N_CORES    # 100352
BF16 = ml_dtypes.bfloat16
N_SLAB = 4       # AllGather slabs per layer (overlap exchange with compute)


def _slabs(w_windows):
    """Split windows into N_SLAB contiguous groups: list of (w0, nwin)."""
    base = w_windows // N_SLAB
    rem = w_windows % N_SLAB
    out = []
    w0 = 0
    for s in range(N_SLAB):
        nwin = base + (1 if s < rem else 0)
        if nwin > 0:
            out.append((w0, nwin))
        w0 += nwin
    return out


# ---------------------------------------------------------------- host prep
def _pack_nodes(cost, sub_real):
    """Assign each node to one of N_CORES*W_WINDOWS bins.

    cost[n] = real in-edges of node n (self-loops ride a dense subtile).
    Caps: 128 nodes per bin, sub_real*128 edge slots per bin.
    Returns bin id per node, or None if packing failed.
    """
    nb = N_CORES * W_WINDOWS
    cap = sub_real * P
    order = np.argsort(-cost, kind="stable")
    bin_load = np.zeros(nb, dtype=np.int64)
    bin_cnt = np.zeros(nb, dtype=np.int64)
    node_bin = np.full(len(cost), -1, dtype=np.int64)
    # greedy: put next (largest) item into currently least-loaded open bin
    heap = [(0, b) for b in range(nb)]
    heapq.heapify(heap)
    stash = []
    for n in order:
        c = cost[n]
        stash.clear()
        placed = False
        while heap:
            load, b = heapq.heappop(heap)
            if bin_load[b] + c <= cap and bin_cnt[b] < P:
                bin_load[b] += c
                bin_cnt[b] += 1
                node_bin[n] = b
                if bin_cnt[b] < P:
                    heapq.heappush(heap, (bin_load[b], b))
                placed = True
                break
            elif bin_cnt[b] < P:
                stash.append((load, b))
        for it in stash:
            heapq.heappush(heap, it)
        if not placed:
            return None, None
    return node_bin, bin_load


def _preprocess(x, edge_index, batch):
    src = np.asarray(edge_index[0], dtype=np.int64)
    dst = np.asarray(edge_index[1], dtype=np.int64)
    batch = np.asarray(batch, dtype=np.int64)
    n = x.shape[0]

    indeg = np.bincount(dst, minlength=n).astype(np.int64)
    deg = indeg.astype(np.float64) + 1.0
    dinv = 1.0 / np.sqrt(deg)
    enorm = (dinv[src] * dinv[dst]).astype(np.float32)
    selfw = (dinv * dinv).astype(np.float32)

    cost = indeg
    for sub in (4, 5, 6):
        node_bin, bin_load = _pack_nodes(cost, sub)
        if node_bin is not None:
            break
    assert node_bin is not None, "window packing failed even at SUB=6"

    # deal bins to cores balanced by load: sort desc, snake over cores
    nb = N_CORES * W_WINDOWS
    order = np.argsort(-bin_load, kind="stable")
    bin_core = np.empty(nb, dtype=np.int64)
    bin_win = np.empty(nb, dtype=np.int64)
    for i, b in enumerate(order):
        rnd, k = divmod(i, N_CORES)
        c = k if rnd % 2 == 0 else N_CORES - 1 - k
        bin_core[b] = c
        bin_win[b] = rnd

    # global slot per node: core*SLOTS + win*P + slot_in_window
    node_core = bin_core[node_bin]
    node_win = bin_win[node_bin]
    # slot within window: stable order by (core, win) groups
    gkey = node_core * W_WINDOWS + node_win
    sort_idx = np.argsort(gkey, kind="stable")
    gsorted = gkey[sort_idx]
    grp_start = np.searchsorted(gsorted, np.arange(nb))
    slot_in_win = np.empty(n, dtype=np.int64)
    slot_in_win[sort_idx] = np.arange(n) - grp_start[gsorted]
    assert slot_in_win.max() < P
    node_slot = node_core * SLOTS + node_win * P + slot_in_win  # global slot id

    # global row in the slab-ordered agout buffer:
    # slab s holds [cores x slab-windows]: row = slab_off + c*rows_s
    #   + (w - w0_s)*P + p
    slabs = _slabs(W_WINDOWS)
    win_slab = np.zeros(W_WINDOWS, dtype=np.int64)
    win_off = np.zeros(W_WINDOWS, dtype=np.int64)   # w - w0_s
    slab_off = np.zeros(N_SLAB, dtype=np.int64)
    slab_rows = np.zeros(N_SLAB, dtype=np.int64)
    off = 0
    for s, (w0, nwin) in enumerate(slabs):
        win_slab[w0:w0 + nwin] = s
        win_off[w0:w0 + nwin] = np.arange(nwin)
        slab_off[s] = off
        slab_rows[s] = nwin * P
        off += N_CORES * nwin * P
    node_grow = (slab_off[win_slab[node_win]]
                 + node_core * slab_rows[win_slab[node_win]]
                 + win_off[node_win] * P + slot_in_win)

    # ---- real-edge lists per (core, window); self loops ride a dense subtile
    e_src_slot = node_grow[src]
    e_wgt = enorm
    e_dst_slot = node_slot[dst]
    e_core = e_dst_slot // SLOTS
    e_win = (e_dst_slot % SLOTS) // P
    e_ploc = e_dst_slot % P

    ekey = e_core * W_WINDOWS + e_win
    es = np.argsort(ekey, kind="stable")
    eks = ekey[es]
    egrp_start = np.searchsorted(eks, np.arange(nb))
    e_rank = np.empty(len(ekey), dtype=np.int64)
    e_rank[es] = np.arange(len(ekey)) - egrp_start[eks]
    assert e_rank.max() < sub * P, "window overflow"

    # slot layout inside window: edge k -> subtile j=k//P, partition p=k%P
    e_j = e_rank // P
    e_p = e_rank % P
    # wval/dloc have sub+1 columns per window (last = self subtile);
    # gidx only sub (self subtile is a dense read of the local shard).
    nsc = sub + 1
    gidx = np.zeros((N_CORES, P, W_WINDOWS * sub), dtype=np.int32)
    wval = np.zeros((N_CORES, P, W_WINDOWS * nsc), dtype=np.float32)
    dloc = np.zeros((N_CORES, P, W_WINDOWS * nsc), dtype=np.float32)
    gidx[e_core, e_p, e_win * sub + e_j] = e_src_slot.astype(np.int32)
    col = e_win * nsc + e_j
    wval[e_core, e_p, col] = e_wgt
    dloc[e_core, e_p, col] = e_ploc.astype(np.float32)
    # self subtile: partition p holds the h-row of slot p -> dloc = p
    scol = np.arange(W_WINDOWS) * nsc + sub
    dloc[:, :, scol] = np.arange(P, dtype=np.float32)[:, None]
    wval.reshape(N_CORES, P, W_WINDOWS, nsc)[
        node_core, slot_in_win, node_win, sub] = selfw

    # ---- per-core node-side arrays (x pre-transposed: [d0, slots] per core)
    d0 = x.shape[1]
    x0 = np.zeros((N_CORES, SLOTS, d0), dtype=BF16)
    x0.reshape(SLOTS_ALL, d0)[node_slot] = x.astype(BF16)
    x0 = np.ascontiguousarray(x0.transpose(0, 2, 1))

    poolP = np.zeros((N_CORES, P, W_WINDOWS * N_GRAPHS), dtype=BF16)
    pc = node_win * N_GRAPHS + batch
    poolP[node_core, slot_in_win, pc] = 1.0

    cnts = np.bincount(batch, minlength=N_GRAPHS).astype(np.float32)
    inv_cnt = (1.0 / np.maximum(cnts, 1.0)).reshape(N_GRAPHS, 1)

    return dict(sub=sub, gidx=gidx, wval=wval, dloc=dloc, x0=x0,
                poolP=poolP, inv_cnt=inv_cnt)


# ---------------------------------------------------------------- device IR
def build_program(sub, has_bias, n_cores=N_CORES, w_windows=W_WINDOWS,
                  dims=DIMS, n_graphs=N_GRAPHS, dbg=False):
    from contextlib import ExitStack

    import concourse.bass as bass
    import concourse.tile as tile
    from concourse import bacc, mybir
    from concourse.masks import make_identity

    dt = mybir.dt
    f32, bf16, i32 = dt.float32, dt.bfloat16, dt.int32
    AF = mybir.ActivationFunctionType
    ALU = mybir.AluOpType
    W = w_windows
    slots = W * P
    slots_all = slots * n_cores
    ncols = W * sub            # gather-index columns
    nsc = sub + 1              # S-matrix columns per window (last = self)
    G = n_graphs
    d_last = dims[-1][1]
    rg = [list(range(n_cores))]

    nc = bacc.Bacc("TRN2", target_bir_lowering=False, debug=False,
                   num_devices=n_cores)

    # ---- I/O
    x0 = nc.dram_tensor("x0", [dims[0][0], slots], bf16, kind="ExternalInput")
    gidx_d = nc.dram_tensor("gidx", [P, ncols], i32, kind="ExternalInput")
    wval_d = nc.dram_tensor("wval", [P, W * nsc], f32, kind="ExternalInput")
    dloc_d = nc.dram_tensor("dloc", [P, W * nsc], f32, kind="ExternalInput")
    W_d = [nc.dram_tensor(f"W{i+1}", [di, do], bf16, kind="ExternalInput")
           for i, (di, do) in enumerate(dims)]
    B_d = [nc.dram_tensor(f"B{i+1}", [P, do], f32, kind="ExternalInput")
           for i, (_, do) in enumerate(dims)]
    poolP_d = nc.dram_tensor("poolP", [P, W * G], bf16, kind="ExternalInput")
    Wl1_d = nc.dram_tensor("Wl1", [d_last, 32], f32, kind="ExternalInput")
    bl1_d = nc.dram_tensor("bl1", [32, 1], f32, kind="ExternalInput")
    Wl_d = nc.dram_tensor("Wl", [32, 2], f32, kind="ExternalInput")
    bl_d = nc.dram_tensor("bl", [2, 1], f32, kind="ExternalInput")
    invc_d = nc.dram_tensor("invc", [G, 1], f32, kind="ExternalInput")
    out_head = nc.dram_tensor("out_head", [2, G], f32, kind="ExternalOutput")

    # ---- internal DRAM
    agin = [nc.dram_tensor(f"agin{l}", [slots, do], bf16)
            for l, (_, do) in enumerate(dims)]
    agout = [nc.dram_tensor(f"agout{l}", [slots_all, do], bf16,
                            addr_space="Shared")
             for l, (_, do) in enumerate(dims)]
    feat = [nc.dram_tensor(f"feat{l}", [slots, do], bf16)
            for l, (_, do) in enumerate(dims[:-1])]
    pool_in = nc.dram_tensor("pool_in", [G, d_last], f32)
    pool_out = nc.dram_tensor("pool_out", [G, d_last], f32,
                              addr_space="Shared")

    with tile.TileContext(nc) as tc, ExitStack() as ctx:
        const = ctx.enter_context(tc.tile_pool(name="const", bufs=1))
        xt_pool = ctx.enter_context(tc.tile_pool(name="xt", bufs=8))
        g_pool = ctx.enter_context(tc.tile_pool(name="g", bufs=32))
        gs_pool = ctx.enter_context(tc.tile_pool(name="gs", bufs=8))
        s_pool = ctx.enter_context(tc.tile_pool(name="s", bufs=24))
        h_pool = ctx.enter_context(tc.tile_pool(name="h", bufs=8))
        psum_m = ctx.enter_context(tc.tile_pool(name="pm", bufs=2, space="PSUM"))
        psum_a = ctx.enter_context(tc.tile_pool(name="pa", bufs=4, space="PSUM"))
        psum_s = ctx.enter_context(tc.tile_pool(name="ps", bufs=2, space="PSUM"))

        # resident constants
        gidx_sb = const.tile([P, ncols], i32, name="gidx_sb")
        nc.sync.dma_start(gidx_sb[:], gidx_d.ap())
        wval_sb = const.tile([P, W * nsc], f32, name="wval_sb")
        nc.sync.dma_start(wval_sb[:], wval_d.ap())
        dloc_sb = const.tile([P, W * nsc], f32, name="dloc_sb")
        nc.sync.dma_start(dloc_sb[:], dloc_d.ap())

        W_sb = []
        for l, (di, do) in enumerate(dims):
            ks = di // P
            t = const.tile([P, ks, do], bf16, name=f"W{l}_sb")
            nc.sync.dma_start(t[:], W_d[l].ap().rearrange(
                "(kt p) do -> p kt do", p=P))
            W_sb.append(t)
        B_sb = []
        for l, (_, do) in enumerate(dims):
            if has_bias[l]:
                t = const.tile([P, do], f32, name=f"B{l}_sb")
                nc.sync.dma_start(t[:], B_d[l].ap())
                B_sb.append(t)
            else:
                B_sb.append(None)

        iota_i = const.tile([P, P], i32, name="iota_i")
        nc.gpsimd.iota(iota_i[:], pattern=[[1, P]], base=0,
                       channel_multiplier=0)
        iota_f = const.tile([P, P], f32, name="iota_f")
        nc.vector.tensor_copy(iota_f[:], iota_i[:])

        poolP_sb = const.tile([P, W * G], bf16, name="poolP_sb")
        nc.sync.dma_start(poolP_sb[:], poolP_d.ap())
        feat4_sb = const.tile([P, W * d_last], bf16, name="feat4_sb")

        Wl1_sb = const.tile([d_last, 32], f32, name="Wl1_sb")
        nc.sync.dma_start(Wl1_sb[:], Wl1_d.ap())
        bl1_sb = const.tile([32, 1], f32, name="bl1_sb")
        nc.sync.dma_start(bl1_sb[:], bl1_d.ap())
        Wl_sb = const.tile([32, 2], f32, name="Wl_sb")
        nc.sync.dma_start(Wl_sb[:], Wl_d.ap())
        bl_sb = const.tile([2, 1], f32, name="bl_sb")
        nc.sync.dma_start(bl_sb[:], bl_d.ap())
        invc_sb = const.tile([G, 1], f32, name="invc_sb")
        nc.sync.dma_start(invc_sb[:], invc_d.ap())

        x0_v = x0.ap().rearrange("(kt p) s -> p kt s", p=P)
        nlay = len(dims)

        def emit_m_window(l, w):
            """h_l[w] = x_{l-1}[w] @ W_l  -> agin[l] rows of window w."""
            di, do = dims[l]
            ks = di // P
            xt = xt_pool.tile([P, ks, P], bf16, tag="xt")
            if l == 0:
                nc.sync.dma_start(xt[:], x0_v[:, :, w * P:(w + 1) * P])
            else:
                nc.sync.dma_start_transpose(
                    xt[:], feat[l - 1].ap()[w * P:(w + 1) * P, :])
            ps = psum_m.tile([P, do], f32, tag="pm")
            for kt in range(ks):
                nc.tensor.matmul(ps[:], lhsT=xt[:, kt, :],
                                 rhs=W_sb[l][:, kt, :],
                                 start=(kt == 0), stop=(kt == ks - 1))
            hm = h_pool.tile([P, do], bf16, tag="hm")
            nc.vector.tensor_copy(hm[:], ps[:])
            nc.scalar.dma_start(agin[l].ap()[w * P:(w + 1) * P, :], hm[:])

        def emit_ag_slab(l, w0s, nwin, goff):
            rows = nwin * P
            nc.gpsimd.collective_compute(
                "AllGather", mybir.AluOpType.bypass, replica_groups=rg,
                ins=[agin[l].ap()[w0s * P:w0s * P + rows, :]],
                outs=[agout[l].ap()[goff:goff + n_cores * rows, :]])

        def emit_a_window(l, w):
            """aggregate + combine window w of layer l."""
            do = dims[l][1]
            ps = psum_a.tile([P, do], f32, tag="pa")
            for ji, j in enumerate(list(range(sub)) + [sub]):
                c = w * nsc + j
                if j < sub:
                    g = g_pool.tile([P, do], bf16, tag="g")
                    nc.gpsimd.indirect_dma_start(
                        out=g[:], out_offset=None, in_=agout[l].ap(),
                        in_offset=bass.IndirectOffsetOnAxis(
                            ap=gidx_sb[:, w * sub + j:w * sub + j + 1],
                            axis=0))
                else:  # self subtile: own shard rows, dense
                    g = gs_pool.tile([P, do], bf16, tag="gs")
                    nc.sync.dma_start(
                        g[:], agin[l].ap()[w * P:(w + 1) * P, :])
                st = s_pool.tile([P, P], bf16, tag="st")
                nc.vector.tensor_scalar(
                    out=st[:], in0=iota_f[:],
                    scalar1=dloc_sb[:, c:c + 1],
                    scalar2=wval_sb[:, c:c + 1],
                    op0=ALU.is_equal, op1=ALU.mult)
                nc.tensor.matmul(ps[:], lhsT=st[:], rhs=g[:],
                                 start=(ji == 0), stop=(ji == sub))
            if has_bias[l]:
                nc.vector.tensor_tensor(out=ps[:], in0=ps[:],
                                        in1=B_sb[l][:], op=ALU.add)
            if l < nlay - 1:
                do_ = do
                ft = h_pool.tile([P, do_], bf16, tag=f"ft{do_}")
                nc.scalar.activation(ft[:], ps[:], AF.Relu)
                nc.scalar.dma_start(feat[l].ap()[w * P:(w + 1) * P, :],
                                    ft[:])
            else:
                nc.vector.tensor_copy(
                    feat4_sb[:, w * d_last:(w + 1) * d_last], ps[:])

        # ---- software pipeline over layers:
        # M(0) runs standalone; each AG slab fires as soon as its windows'
        # agin rows exist; A(l) windows interleave with M(l+1) windows so
        # AG(l+1) slabs launch mid-A(l) and overlap with the gather stream.
        slabs = _slabs(W)
        goffs = []
        goff = 0
        for (w0s, nwin) in slabs:
            goffs.append(goff)
            goff += n_cores * nwin * P
        for si, (w0s, nwin) in enumerate(slabs):
            for w in range(w0s, w0s + nwin):
                emit_m_window(0, w)
            emit_ag_slab(0, w0s, nwin, goffs[si])
        # M(l+1) windows trail A(l) windows by LAG so every cross-engine
        # dependency is stale by the time it reaches a queue head (otherwise
        # the DVE cast + PE chain advance in lockstep, one window per period).
        LAG = min(2, W - 1)
        slab_last = {w0s + nwin - 1: (si, w0s, nwin)
                     for si, (w0s, nwin) in enumerate(slabs)}

        def emit_m_and_ag(l1, wm):
            emit_m_window(l1, wm)
            if wm in slab_last:
                si, w0s, nwin = slab_last[wm]
                emit_ag_slab(l1, w0s, nwin, goffs[si])

        for l in range(nlay):
            for w in range(W):
                emit_a_window(l, w)
                if l + 1 < nlay and w >= LAG:
                    emit_m_and_ag(l + 1, w - LAG)
            if l + 1 < nlay:
                for wm in range(W - LAG, W):
                    emit_m_and_ag(l + 1, wm)

        # ---- mean pool
        pp = psum_s.tile([G, d_last], f32, name="pool_ps", tag="ps_small")
        for w in range(W):
            nc.tensor.matmul(pp[:], lhsT=poolP_sb[:, w * G:(w + 1) * G],
                             rhs=feat4_sb[:, w * d_last:(w + 1) * d_last],
                             start=(w == 0), stop=(w == W - 1))
        pool_sb = const.tile([G, d_last], f32, name="pool_sb")
        nc.vector.tensor_copy(pool_sb[:], pp[:])
        nc.sync.dma_start(pool_in.ap(), pool_sb[:])
        nc.gpsimd.collective_compute(
            "AllReduce", mybir.AluOpType.add, replica_groups=rg,
            ins=[pool_in.ap()], outs=[pool_out.ap()])
        psum_sb = const.tile([G, d_last], f32, name="psum_sb")
        nc.sync.dma_start(psum_sb[:], pool_out.ap())
        pooled = const.tile([G, d_last], f32, name="pooled")
        nc.vector.tensor_scalar_mul(pooled[:], psum_sb[:], invc_sb[:, :1])

        # ---- head (every core computes the same result)
        iden = const.tile([G, G], f32, name="iden")
        make_identity(nc, iden[:])
        pt_ps = psum_s.tile([d_last, G], f32, name="pt_ps", tag="ps_small")
        nc.tensor.transpose(pt_ps[:], pooled[:], iden[:])
        pt = const.tile([d_last, G], f32, name="pt")
        nc.vector.tensor_copy(pt[:], pt_ps[:])
        ps1 = psum_s.tile([32, G], f32, name="ps1", tag="ps_small")
        nc.tensor.matmul(ps1[:], lhsT=Wl1_sb[:], rhs=pt[:])
        h1 = const.tile([32, G], f32, name="h1")
        nc.scalar.activation(h1[:], ps1[:], AF.Relu, bias=bl1_sb[:, :1])
        ps2 = psum_s.tile([2, G], f32, name="ps2", tag="ps_small")
        nc.tensor.matmul(ps2[:], lhsT=Wl_sb[:], rhs=h1[:])
        oh = const.tile([2, G], f32, name="oh")
        nc.vector.tensor_scalar_add(oh[:], ps2[:], bl_sb[:, :1])
        nc.sync.dma_start(out_head.ap(), oh[:])

        if dbg:
            d_agin0 = nc.dram_tensor("d_agin0", [slots, dims[0][1]], bf16,
                                     kind="ExternalOutput")
            nc.sync.dma_start(d_agin0.ap(), agin[0].ap())
            d_agout0 = nc.dram_tensor("d_agout0", [slots_all, dims[0][1]],
                                      bf16, kind="ExternalOutput")
            nc.sync.dma_start(d_agout0.ap(), agout[0].ap())
            d_feat0 = nc.dram_tensor("d_feat0", [slots, dims[0][1]], bf16,
                                     kind="ExternalOutput")
            nc.sync.dma_start(d_feat0.ap(), feat[0].ap())
            d_feat4 = nc.dram_tensor("d_feat4", [P, W * d_last], bf16,
                                     kind="ExternalOutput")
            nc.sync.dma_start(d_feat4.ap(), feat4_sb[:])
            d_pool = nc.dram_tensor("d_pool", [G, d_last], f32,
                                    kind="ExternalOutput")
            nc.sync.dma_start(d_pool.ap(), psum_sb[:])

    nc.compile()
    return nc


# ---------------------------------------------------------------- entry
_CACHE = {}


def _make_in_maps(prep, inp):
    Ws = [np.asarray(inp[f"W{i+1}"]) for i in range(4)]
    bs = [np.asarray(inp[f"b{i+1}"]) for i in range(4)]
    in_maps = []
    for c in range(N_CORES):
        m = dict(
            x0=prep["x0"][c],
            gidx=prep["gidx"][c], wval=prep["wval"][c], dloc=prep["dloc"][c],
            poolP=prep["poolP"][c], invc=prep["inv_cnt"],
            Wl1=np.asarray(inp["Wl1"], np.float32),
            bl1=np.asarray(inp["bl1"], np.float32).reshape(-1, 1),
            Wl=np.asarray(inp["Wl"], np.float32),
            bl=np.asarray(inp["bl"], np.float32).reshape(-1, 1),
        )
        for i, (wm, bv) in enumerate(zip(Ws, bs)):
            m[f"W{i+1}"] = wm.astype(BF16)
            m[f"B{i+1}"] = np.broadcast_to(
                np.asarray(bv, np.float32), (P, len(bv))).copy()
        in_maps.append(m)
    return in_maps


def kernel(x, edge_index, batch, W1, b1, W2, b2, W3, b3, W4, b4,
           Wl1, bl1, Wl, bl):
    from concourse import bass_utils

    x = np.asarray(x)
    prep = _preprocess(x, np.asarray(edge_index), np.asarray(batch))
    sub = prep["sub"]
    bs = [np.asarray(b) for b in (b1, b2, b3, b4)]
    has_bias = tuple(bool(np.any(b != 0)) for b in bs)

    key = (sub, has_bias)
    if key not in _CACHE:
        _CACHE[key] = build_program(sub, has_bias)
    nc = _CACHE[key]

    inp = dict(W1=W1, b1=b1, W2=W2, b2=b2, W3=W3, b3=b3, W4=W4, b4=b4,
               Wl1=Wl1, bl1=bl1, Wl=Wl, bl=bl)
    in_maps = _make_in_maps(prep, inp)
    res = bass_utils.run_bass_kernel_spmd(
        nc, in_maps, core_ids=list(range(N_CORES)))
    out = res.results[0]["out_head"]
    return np.ascontiguousarray(out.T.astype(np.float32))



# revision 3
# speedup vs baseline: 1.0958x; 1.0958x over previous
"""GCN (4x GCNConv + global mean pool + MLP head) on 8 Trainium2 NeuronCores.

Strategy (node sharding, v2 — batched dma_gather + host-built one-hot S):
  - Host: relabel the 100k nodes into 8 cores x 98 windows x 128 slots via a
    capacity-constrained bin packing. Edges are grouped per (window, range)
    where the 5 ranges partition the global row space into int16-addressable
    chunks (dma_gather indices are int16); each group is padded to a multiple
    of 128 "column" slots. Per (core, window, column) the structure is:
      gidx  [128, C*8] int16 — wrapped gather indices (k -> [k%16, k//16],
             replicated across the 8 GpSimd core groups), range-relative
      S     [128, C*128] fp8(e4m3) — exact one-hot scatter matrices
             (S[p, c*128+d] = 1 iff edge slot (c, p) targets local dst d)
    The GCN normalization D^-1/2 (A+I) D^-1/2 is folded into per-node scales:
    sources are pre-scaled by dinv (on host for x, via the M-step output cast
    on device for h), and the per-dst dinv is applied by the post-aggregation
    activation's per-partition scale. Self-loops become one constant identity
    matmul per window against the dense local rows.
  - Layer 1 aggregates FIRST (out1 = (A~ x) @ W1): x is replicated input, so
    the gathers read a host-staged x_full (no AllGather for the 512-wide h1).
    agg1 round-trips DRAM once to obtain its transpose (xbar transposed read)
    for the W1 matmul.
  - Layers 2..4: M-step (feat @ W, via xbar-transposed reads of feat),
    AllGather of the h' shard per slab (4 slabs), then batched aggregation:
    one dma_gather per (batch of ~5-7 windows, range) fetches all edge rows
    (SWDGE cost 994ns + 0.34ns/row — batching is ~10x cheaper than the
    per-column indirect DMAs), then S matmuls accumulate in PSUM.
  - Mean pool: PE matmuls P_w^T @ feat4_w, AllReduce, tiny MLP head.
"""

import heapq

import numpy as np
import ml_dtypes

# ---------------------------------------------------------------- constants
N_NODES = 100000
N_EDGES = 400000
N_GRAPHS = 64
DIMS = [(512, 512), (512, 256), (256, 128), (128, 64)]
N_CORES = 8
P = 128          # partitions / slots per window
W_WINDOWS = 98   # windows per core -> 12544 slots/core
SLOTS = W_WINDOWS * P          # 12544
SLOTS_ALL = SLOTS * N_CORES    # 100352
BF16 = ml_dtypes.bfloat16
FP8 = ml_dtypes.float8_e4m3
N_SLAB = 4       # AllGather slabs per layer
NR = 5           # gather address ranges (int16 index limit)
NB_L1 = 5        # windows per batch, layer 1 (512-wide gather tiles)
NB = 7           # windows per batch, layers 2-4
LAG = 2          # batches M(l+1) trails A(l) by
D4P = 128        # layer-4 padded width (gather rows must be %256B)


def _slabs(w_windows):
    base = w_windows // N_SLAB
    rem = w_windows % N_SLAB
    out = []
    w0 = 0
    for s in range(N_SLAB):
        nwin = base + (1 if s < rem else 0)
        if nwin > 0:
            out.append((w0, nwin))
        w0 += nwin
    return out


def _ranges():
    base = SLOTS_ALL // NR
    rem = SLOTS_ALL % NR
    sizes = [base + (1 if r < rem else 0) for r in range(NR)]
    lo = np.concatenate([[0], np.cumsum(sizes)])
    assert max(sizes) <= 32767
    return lo  # [NR+1]


def _grid(nb, w_windows=W_WINDOWS):
    return [(w0, min(nb, w_windows - w0)) for w0 in range(0, w_windows, nb)]


# ---------------------------------------------------------------- host prep
def _pack_nodes(cost, sub_real):
    nb = N_CORES * W_WINDOWS
    cap = sub_real * P
    order = np.argsort(-cost, kind="stable")
    bin_load = np.zeros(nb, dtype=np.int64)
    bin_cnt = np.zeros(nb, dtype=np.int64)
    node_bin = np.full(len(cost), -1, dtype=np.int64)
    heap = [(0, b) for b in range(nb)]
    heapq.heapify(heap)
    stash = []
    for n in order:
        c = cost[n]
        stash.clear()
        placed = False
        while heap:
            load, b = heapq.heappop(heap)
            if bin_load[b] + c <= cap and bin_cnt[b] < P:
                bin_load[b] += c
                bin_cnt[b] += 1
                node_bin[n] = b
                if bin_cnt[b] < P:
                    heapq.heappush(heap, (bin_load[b], b))
                placed = True
                break
            elif bin_cnt[b] < P:
                stash.append((load, b))
        for it in stash:
            heapq.heappush(heap, it)
        if not placed:
            return None, None
    return node_bin, bin_load


def _preprocess(x, edge_index, batch):
    src = np.asarray(edge_index[0], dtype=np.int64)
    dst = np.asarray(edge_index[1], dtype=np.int64)
    batch = np.asarray(batch, dtype=np.int64)
    n = x.shape[0]

    indeg = np.bincount(dst, minlength=n).astype(np.int64)
    deg = indeg.astype(np.float64) + 1.0
    dinv = 1.0 / np.sqrt(deg)

    for sub in (4, 5, 6):
        node_bin, bin_load = _pack_nodes(indeg, sub)
        if node_bin is not None:
            break
    assert node_bin is not None, "window packing failed even at SUB=6"

    nbins = N_CORES * W_WINDOWS
    order = np.argsort(-bin_load, kind="stable")
    bin_core = np.empty(nbins, dtype=np.int64)
    bin_win = np.empty(nbins, dtype=np.int64)
    for i, b in enumerate(order):
        rnd, k = divmod(i, N_CORES)
        c = k if rnd % 2 == 0 else N_CORES - 1 - k
        bin_core[b] = c
        bin_win[b] = rnd

    node_core = bin_core[node_bin]
    node_win = bin_win[node_bin]
    gkey = node_core * W_WINDOWS + node_win
    sort_idx = np.argsort(gkey, kind="stable")
    gsorted = gkey[sort_idx]
    grp_start = np.searchsorted(gsorted, np.arange(nbins))
    slot_in_win = np.empty(n, dtype=np.int64)
    slot_in_win[sort_idx] = np.arange(n) - grp_start[gsorted]
    assert slot_in_win.max() < P

    # global row in the slab-ordered gather/AllGather buffers
    slabs = _slabs(W_WINDOWS)
    win_slab = np.zeros(W_WINDOWS, dtype=np.int64)
    win_off = np.zeros(W_WINDOWS, dtype=np.int64)
    slab_off = np.zeros(N_SLAB, dtype=np.int64)
    slab_rows = np.zeros(N_SLAB, dtype=np.int64)
    off = 0
    for s, (w0, nwin) in enumerate(slabs):
        win_slab[w0:w0 + nwin] = s
        win_off[w0:w0 + nwin] = np.arange(nwin)
        slab_off[s] = off
        slab_rows[s] = nwin * P
        off += N_CORES * nwin * P
    node_grow = (slab_off[win_slab[node_win]]
                 + node_core * slab_rows[win_slab[node_win]]
                 + win_off[node_win] * P + slot_in_win)

    # ---- edge slots: group per (core, range, window), pad to 128-cols
    r_lo = _ranges()
    g_src = node_grow[src]
    e_range = np.searchsorted(r_lo[1:], g_src, side="right")
    e_core = node_core[dst]
    e_win = node_win[dst]
    e_dstp = slot_in_win[dst]

    cnt = np.zeros((N_CORES, NR, W_WINDOWS), dtype=np.int64)
    np.add.at(cnt, (e_core, e_range, e_win), 1)
    cols_rw = -(-cnt.max(axis=0) // P)          # [NR, W] shared plan
    colbase = np.zeros((NR, W_WINDOWS), dtype=np.int64)
    colbase.ravel()[1:] = np.cumsum(cols_rw.ravel())[:-1]
    total_cols = int(cols_rw.sum())

    key = (e_core * NR + e_range) * W_WINDOWS + e_win
    es = np.argsort(key, kind="stable")
    ks = key[es]
    gstart = np.searchsorted(ks, np.arange(N_CORES * NR * W_WINDOWS))
    e_rank = np.empty(len(key), dtype=np.int64)
    e_rank[es] = np.arange(len(key)) - gstart[ks]
    assert (e_rank < cols_rw[e_range, e_win] * P).all()
    e_col = colbase[e_range, e_win] + e_rank // P
    e_p = e_rank % P

    gidx_flat = np.zeros((N_CORES, total_cols * P), dtype=np.int16)
    gidx_flat[e_core, e_col * P + e_p] = (g_src - r_lo[e_range]).astype(
        np.int16)
    S = np.zeros((N_CORES, P, total_cols * P), dtype=FP8)
    S[e_core, e_p, e_col * P + e_dstp] = 1.0
    # wrapped [16, X] then replicated across the 8 Q7 core groups
    wrapped = gidx_flat.reshape(N_CORES, total_cols * P // 16, 16)
    gidx = np.ascontiguousarray(
        np.tile(wrapped.transpose(0, 2, 1), (1, 8, 1)))  # [8, 128, X]

    dinv_slot = np.ones((N_CORES, P, W_WINDOWS), dtype=np.float32)
    dinv_slot[node_core, slot_in_win, node_win] = dinv

    d0 = x.shape[1]
    xs = np.asarray(x, np.float64) * dinv[:, None]
    x_full = np.zeros((SLOTS_ALL, d0), dtype=BF16)
    x_full[node_grow] = xs.astype(BF16)
    x_self = np.zeros((N_CORES, SLOTS, d0), dtype=BF16)
    x_self.reshape(N_CORES * SLOTS, d0)[
        node_core * SLOTS + node_win * P + slot_in_win] = xs.astype(BF16)

    poolP = np.zeros((N_CORES, P, W_WINDOWS * N_GRAPHS), dtype=BF16)
    pc = node_win * N_GRAPHS + batch
    poolP[node_core, slot_in_win, pc] = 1.0

    cnts = np.bincount(batch, minlength=N_GRAPHS).astype(np.float32)
    inv_cnt = (1.0 / np.maximum(cnts, 1.0)).reshape(N_GRAPHS, 1)

    plan = dict(cols_rw=cols_rw, colbase=colbase, total_cols=total_cols,
                r_lo=r_lo)
    return dict(sub=sub, plan=plan, gidx=gidx, S=S, dinv_slot=dinv_slot,
                x_full=x_full, x_self=x_self, poolP=poolP, inv_cnt=inv_cnt)


# ---------------------------------------------------------------- device IR
def build_program(plan, has_bias, n_cores=N_CORES, w_windows=W_WINDOWS,
                  dims=DIMS, n_graphs=N_GRAPHS):
    from contextlib import ExitStack

    import concourse.bass as bass
    import concourse.tile as tile
    from concourse import bacc, mybir
    from concourse.masks import make_identity

    dt = mybir.dt
    f32, bf16, fp8 = dt.float32, dt.bfloat16, dt.float8e4
    i16 = dt.int16
    AF = mybir.ActivationFunctionType
    ALU = mybir.AluOpType
    W = w_windows
    slots = W * P
    slots_all = slots * n_cores
    G = n_graphs
    d_last = dims[-1][1]
    rg = [list(range(n_cores))]
    nlay = len(dims)
    d0 = dims[0][0]

    cols_rw = plan["cols_rw"]        # [NR, W]
    colbase = plan["colbase"]        # [NR, W]
    total_cols = plan["total_cols"]
    r_lo = plan["r_lo"]              # [NR+1]

    slabs = _slabs(W)
    goffs = []
    goff = 0
    for (w0s, nwin) in slabs:
        goffs.append(goff)
        goff += n_cores * nwin * P
    slab_last = {w0s + nwin - 1: (si, w0s, nwin)
                 for si, (w0s, nwin) in enumerate(slabs)}

    # gather width per layer l (source row width)
    gw = {1: d0, 2: dims[1][1], 3: dims[2][1], 4: D4P}

    nc = bacc.Bacc("TRN2", target_bir_lowering=False, debug=False,
                   num_devices=n_cores)

    # ---- I/O
    xfull_d = nc.dram_tensor("x_full", [slots_all, d0], bf16,
                             kind="ExternalInput")
    xself_d = nc.dram_tensor("x_self", [slots, d0], bf16,
                             kind="ExternalInput")
    gidx_d = nc.dram_tensor("gidx", [P, total_cols * 8], i16,
                            kind="ExternalInput")
    S_d = nc.dram_tensor("S", [P, total_cols * P], fp8, kind="ExternalInput")
    dinv_d = nc.dram_tensor("dinv", [P, W], f32, kind="ExternalInput")
    W_d = [nc.dram_tensor("W1", [d0, dims[0][1]], bf16, kind="ExternalInput"),
           nc.dram_tensor("W2", [dims[1][0], dims[1][1]], bf16,
                          kind="ExternalInput"),
           nc.dram_tensor("W3", [dims[2][0], dims[2][1]], bf16,
                          kind="ExternalInput"),
           nc.dram_tensor("W4", [dims[3][0], D4P], bf16,
                          kind="ExternalInput")]
    B_d = [nc.dram_tensor(f"B{i+1}", [P, do], f32, kind="ExternalInput")
           for i, (_, do) in enumerate(dims)]
    poolP_d = nc.dram_tensor("poolP", [P, W * G], bf16, kind="ExternalInput")
    Wl1_d = nc.dram_tensor("Wl1", [d_last, 32], f32, kind="ExternalInput")
    bl1_d = nc.dram_tensor("bl1", [32, 1], f32, kind="ExternalInput")
    Wl_d = nc.dram_tensor("Wl", [32, 2], f32, kind="ExternalInput")
    bl_d = nc.dram_tensor("bl", [2, 1], f32, kind="ExternalInput")
    invc_d = nc.dram_tensor("invc", [G, 1], f32, kind="ExternalInput")
    out_head = nc.dram_tensor("out_head", [2, G], f32, kind="ExternalOutput")

    # ---- internal DRAM
    agg1 = nc.dram_tensor("agg1", [slots, d0], bf16)
    feat = [None] + [nc.dram_tensor(f"feat{l}", [slots, dims[l - 1][1]], bf16)
                     for l in (1, 2, 3)]
    agin = {l: nc.dram_tensor(f"agin{l}", [slots, gw[l]], bf16)
            for l in (2, 3, 4)}
    agout = {l: nc.dram_tensor(f"agout{l}", [slots_all, gw[l]], bf16,
                               addr_space="Shared")
             for l in (2, 3, 4)}
    pool_in = nc.dram_tensor("pool_in", [G, d_last], f32)
    pool_out = nc.dram_tensor("pool_out", [G, d_last], f32,
                              addr_space="Shared")

    grid1 = _grid(NB_L1)
    grid = _grid(NB)
    grids = {1: grid1, 2: grid, 3: grid, 4: grid}

    # batch gather segments: for (w0, nbw) -> list of
    #   (r, c0, ncols, tile_col_offset)
    def segments(w0, nbw):
        segs = []
        toff = 0
        for r in range(NR):
            c0 = int(colbase[r, w0])
            ncol = int(cols_rw[r, w0:w0 + nbw].sum())
            if ncol > 0:
                segs.append((r, c0, ncol, toff))
                toff += ncol
        return segs, toff

    # per-window matmul columns: (global S col, batch-tile col)
    def win_cols(w0, nbw, wi):
        w = w0 + wi
        out = []
        toff = 0
        for r in range(NR):
            base_b = int(colbase[r, w0])
            ncol_b = int(cols_rw[r, w0:w0 + nbw].sum())
            if ncol_b == 0:
                continue
            cstart = int(colbase[r, w])
            for j in range(int(cols_rw[r, w])):
                out.append((cstart + j, toff + (cstart - base_b) + j))
            toff += ncol_b
        return out

    with tile.TileContext(nc) as tc, ExitStack() as ctx:
        const = ctx.enter_context(tc.tile_pool(name="const", bufs=1))
        g_pool = ctx.enter_context(tc.tile_pool(name="g", bufs=2))
        sl_pool = ctx.enter_context(tc.tile_pool(name="sl", bufs=2))
        xt_pool = ctx.enter_context(tc.tile_pool(name="xt", bufs=2))
        h_pool = ctx.enter_context(tc.tile_pool(name="h", bufs=2))
        psum_a = ctx.enter_context(tc.tile_pool(name="pa", bufs=2,
                                                space="PSUM"))
        psum_m = ctx.enter_context(tc.tile_pool(name="pm", bufs=2,
                                                space="PSUM"))
        psum_s = ctx.enter_context(tc.tile_pool(name="ps", bufs=2,
                                                space="PSUM"))

        # ---- resident constants
        S_sb = const.tile([P, total_cols * P], fp8, name="S_sb")
        nc.sync.dma_start(S_sb[:], S_d.ap())
        gidx_sb = const.tile([P, total_cols * 8], i16, name="gidx_sb")
        nc.sync.dma_start(gidx_sb[:], gidx_d.ap())
        dinv_sb = const.tile([P, W], f32, name="dinv_sb")
        nc.sync.dma_start(dinv_sb[:], dinv_d.ap())

        iden_sb = const.tile([P, P], bf16, name="iden_sb")
        make_identity(nc, iden_sb[:])

        W_sb = []
        for l in range(nlay):
            di = dims[l][0]
            do = D4P if l == 3 else dims[l][1]
            ks = di // P
            t = const.tile([P, ks, do], bf16, name=f"W{l}_sb")
            nc.sync.dma_start(t[:], W_d[l].ap().rearrange(
                "(kt p) do -> p kt do", p=P))
            W_sb.append(t)
        B_sb = []
        for l, (_, do) in enumerate(dims):
            if has_bias[l]:
                t = const.tile([P, do], f32, name=f"B{l}_sb")
                nc.sync.dma_start(t[:], B_d[l].ap())
                B_sb.append(t)
            else:
                B_sb.append(None)

        feat4_sb = const.tile([P, W * d_last], bf16, name="feat4_sb")

        Wl1_sb = const.tile([d_last, 32], f32, name="Wl1_sb")
        nc.sync.dma_start(Wl1_sb[:], Wl1_d.ap())
        bl1_sb = const.tile([32, 1], f32, name="bl1_sb")
        nc.sync.dma_start(bl1_sb[:], bl1_d.ap())
        Wl_sb = const.tile([32, 2], f32, name="Wl_sb")
        nc.sync.dma_start(Wl_sb[:], Wl_d.ap())
        bl_sb = const.tile([2, 1], f32, name="bl_sb")
        nc.sync.dma_start(bl_sb[:], bl_d.ap())
        invc_sb = const.tile([G, 1], f32, name="invc_sb")
        nc.sync.dma_start(invc_sb[:], invc_d.ap())

        def emit_agg(l, b):
            """Aggregate windows of batch b of layer l: psum = S^T g + self."""
            w0, nbw = grids[l][b]
            do_g = gw[l]
            do_l = dims[l - 1][1]
            src = xfull_d if l == 1 else agout[l]
            segs, tcols = segments(w0, nbw)
            g = g_pool.tile([P, tcols, do_g], bf16, tag="g")
            for (r, c0, ncol, toff) in segs:
                nc.gpsimd.dma_gather(
                    out_ap=g[:, toff:toff + ncol, :],
                    in_ap=src.ap()[int(r_lo[r]):int(r_lo[r + 1]), :],
                    idxs_ap=gidx_sb[:, c0 * 8:(c0 + ncol) * 8],
                    num_idxs=ncol * P, num_idxs_reg=ncol * P,
                    elem_size=do_g, single_packet=False)
            selfsrc = xself_d if l == 1 else agin[l]
            sl = sl_pool.tile([P, nbw, do_g], bf16, tag="sl")
            nc.sync.dma_start(
                sl[:], selfsrc.ap()[w0 * P:(w0 + nbw) * P, :].rearrange(
                    "(nb p) d -> p nb d", p=P))
            out_t = None
            if l == 1:
                out_t = h_pool.tile([P, nbw, do_g], bf16, tag="a1")
            elif l < nlay:
                out_t = h_pool.tile([P, nbw, do_l], bf16, tag="ft")
            for wi in range(nbw):
                w = w0 + wi
                ps = psum_a.tile([P, do_l], f32, tag="pa")
                wcols = win_cols(w0, nbw, wi)
                for k, (scol, tcol) in enumerate(wcols):
                    nc.tensor.matmul(ps[:],
                                     lhsT=S_sb[:, scol * P:(scol + 1) * P],
                                     rhs=g[:, tcol, :do_l],
                                     start=(k == 0), stop=False)
                nc.tensor.matmul(ps[:], lhsT=iden_sb[:],
                                 rhs=sl[:, wi, :do_l],
                                 start=(len(wcols) == 0), stop=True)
                if l == 1:
                    nc.scalar.activation(out_t[:, wi, :], ps[:], AF.Copy,
                                         scale=dinv_sb[:, w:w + 1])
                elif l < nlay:
                    if has_bias[l - 1]:
                        nc.vector.scalar_tensor_tensor(
                            out=ps[:], in0=ps[:],
                            scalar=dinv_sb[:, w:w + 1], in1=B_sb[l - 1][:],
                            op0=ALU.mult, op1=ALU.add)
                        nc.scalar.activation(out_t[:, wi, :], ps[:], AF.Relu)
                    else:
                        nc.scalar.activation(out_t[:, wi, :], ps[:], AF.Relu,
                                             scale=dinv_sb[:, w:w + 1])
                else:
                    if has_bias[l - 1]:
                        nc.vector.scalar_tensor_tensor(
                            out=ps[:], in0=ps[:],
                            scalar=dinv_sb[:, w:w + 1], in1=B_sb[l - 1][:],
                            op0=ALU.mult, op1=ALU.add)
                        nc.scalar.activation(
                            feat4_sb[:, w * d_last:(w + 1) * d_last], ps[:],
                            AF.Copy)
                    else:
                        nc.scalar.activation(
                            feat4_sb[:, w * d_last:(w + 1) * d_last], ps[:],
                            AF.Copy, scale=dinv_sb[:, w:w + 1])
            if l == 1:
                nc.scalar.dma_start(
                    agg1.ap()[w0 * P:(w0 + nbw) * P, :].rearrange(
                        "(nb p) d -> p nb d", p=P), out_t[:])
                xt = xt_pool.tile([P, d0 // P, nbw * P], bf16, tag="xt1")
                nc.sync.dma_start_transpose(
                    xt[:], agg1.ap()[w0 * P:(w0 + nbw) * P, :])
                return xt
            if l < nlay:
                nc.scalar.dma_start(
                    feat[l].ap()[w0 * P:(w0 + nbw) * P, :].rearrange(
                        "(nb p) d -> p nb d", p=P), out_t[:])
            return None

        def emit_h1(b, xt):
            """out1 = (A~x) @ W1 (+b1), ReLU -> feat1, for batch b of grid1."""
            w0, nbw = grid1[b]
            do = dims[0][1]
            ks = d0 // P
            f1 = h_pool.tile([P, nbw, do], bf16, tag="f1")
            for wi in range(nbw):
                psh = psum_m.tile([P, do], f32, tag="pm")
                for kt in range(ks):
                    nc.tensor.matmul(psh[:],
                                     lhsT=xt[:, kt, wi * P:(wi + 1) * P],
                                     rhs=W_sb[0][:, kt, :],
                                     start=(kt == 0), stop=(kt == ks - 1))
                if has_bias[0]:
                    nc.vector.tensor_tensor(out=psh[:], in0=psh[:],
                                            in1=B_sb[0][:], op=ALU.add)
                nc.scalar.activation(f1[:, wi, :], psh[:], AF.Relu)
            nc.scalar.dma_start(
                feat[1].ap()[w0 * P:(w0 + nbw) * P, :].rearrange(
                    "(nb p) d -> p nb d", p=P), f1[:])

        def emit_m(l, b):
            """M-step of layer l (2..4): agin_l = dinv * (feat_{l-1} @ W_l);
            fires AG slabs when due."""
            w0, nbw = grids[l - 1][b]
            di = dims[l - 1][0]
            do = gw[l]
            ks = di // P
            xt = xt_pool.tile([P, ks, nbw * P], bf16, tag="xtm")
            nc.sync.dma_start_transpose(
                xt[:], feat[l - 1].ap()[w0 * P:(w0 + nbw) * P, :])
            hm = h_pool.tile([P, nbw, do], bf16, tag="hm")
            for wi in range(nbw):
                w = w0 + wi
                psm = psum_m.tile([P, do], f32, tag="pm")
                for kt in range(ks):
                    nc.tensor.matmul(psm[:],
                                     lhsT=xt[:, kt, wi * P:(wi + 1) * P],
                                     rhs=W_sb[l - 1][:, kt, :],
                                     start=(kt == 0), stop=(kt == ks - 1))
                nc.vector.tensor_scalar_mul(hm[:, wi, :], psm[:],
                                            dinv_sb[:, w:w + 1])
            nc.scalar.dma_start(
                agin[l].ap()[w0 * P:(w0 + nbw) * P, :].rearrange(
                    "(nb p) d -> p nb d", p=P), hm[:])
            for w in range(w0, w0 + nbw):
                if w in slab_last:
                    si, w0s, nwin = slab_last[w]
                    rows = nwin * P
                    nc.gpsimd.collective_compute(
                        "AllGather", mybir.AluOpType.bypass,
                        replica_groups=rg,
                        ins=[agin[l].ap()[w0s * P:w0s * P + rows, :]],
                        outs=[agout[l].ap()[goffs[si]:
                                            goffs[si] + n_cores * rows, :]])

        # ---- layer 1: aggregate-first; h1 one batch behind; M(2) trails
        xts = {}
        for b in range(len(grid1)):
            xts[b] = emit_agg(1, b)
            if b >= 1:
                emit_h1(b - 1, xts.pop(b - 1))
            if b >= 1 + LAG:
                emit_m(2, b - 1 - LAG)
        emit_h1(len(grid1) - 1, xts.pop(len(grid1) - 1))
        for b in range(max(0, len(grid1) - 1 - LAG), len(grid1)):
            emit_m(2, b)

        # ---- layers 2..4
        for l in (2, 3, 4):
            gl = grids[l]
            for b in range(len(gl)):
                emit_agg(l, b)
                if l < nlay and b >= LAG:
                    emit_m(l + 1, b - LAG)
            if l < nlay:
                for b in range(max(0, len(gl) - LAG), len(gl)):
                    emit_m(l + 1, b)

        # ---- mean pool (poolP loaded late into a gather-pool slot)
        poolP_sb = g_pool.tile([P, W * G], bf16, tag="g")
        nc.sync.dma_start(poolP_sb[:], poolP_d.ap())
        pp = psum_s.tile([G, d_last], f32, name="pool_ps", tag="ps_small")
        for w in range(W):
            nc.tensor.matmul(pp[:], lhsT=poolP_sb[:, w * G:(w + 1) * G],
                             rhs=feat4_sb[:, w * d_last:(w + 1) * d_last],
                             start=(w == 0), stop=(w == W - 1))
        pool_sb = const.tile([G, d_last], f32, name="pool_sb")
        nc.vector.tensor_copy(pool_sb[:], pp[:])
        nc.sync.dma_start(pool_in.ap(), pool_sb[:])
        nc.gpsimd.collective_compute(
            "AllReduce", mybir.AluOpType.add, replica_groups=rg,
            ins=[pool_in.ap()], outs=[pool_out.ap()])
        psum_sb = const.tile([G, d_last], f32, name="psum_sb")
        nc.sync.dma_start(psum_sb[:], pool_out.ap())
        pooled = const.tile([G, d_last], f32, name="pooled")
        nc.vector.tensor_scalar_mul(pooled[:], psum_sb[:], invc_sb[:, :1])

        # ---- head (every core computes the same result)
        ideng = const.tile([G, G], f32, name="ideng")
        make_identity(nc, ideng[:])
        pt_ps = psum_s.tile([d_last, G], f32, name="pt_ps", tag="ps_small")
        nc.tensor.transpose(pt_ps[:], pooled[:], ideng[:])
        pt = const.tile([d_last, G], f32, name="pt")
        nc.vector.tensor_copy(pt[:], pt_ps[:])
        ps1 = psum_s.tile([32, G], f32, name="ps1", tag="ps_small")
        nc.tensor.matmul(ps1[:], lhsT=Wl1_sb[:], rhs=pt[:])
        h1 = const.tile([32, G], f32, name="h1")
        nc.scalar.activation(h1[:], ps1[:], AF.Relu, bias=bl1_sb[:, :1])
        ps2 = psum_s.tile([2, G], f32, name="ps2", tag="ps_small")
        nc.tensor.matmul(ps2[:], lhsT=Wl_sb[:], rhs=h1[:])
        oh = const.tile([2, G], f32, name="oh")
        nc.vector.tensor_scalar_add(oh[:], ps2[:], bl_sb[:, :1])
        nc.sync.dma_start(out_head.ap(), oh[:])

    nc.compile()
    return nc


# ---------------------------------------------------------------- entry
_CACHE = {}


def _make_in_maps(prep, inp):
    Ws = [np.asarray(inp[f"W{i+1}"]) for i in range(4)]
    bs = [np.asarray(inp[f"b{i+1}"]) for i in range(4)]
    W4p = np.zeros((DIMS[3][0], D4P), dtype=BF16)
    W4p[:, :DIMS[3][1]] = Ws[3].astype(BF16)
    in_maps = []
    for c in range(N_CORES):
        m = dict(
            x_full=prep["x_full"],
            x_self=prep["x_self"][c],
            gidx=prep["gidx"][c], S=prep["S"][c],
            dinv=np.ascontiguousarray(prep["dinv_slot"][c]),
            poolP=prep["poolP"][c], invc=prep["inv_cnt"],
            Wl1=np.asarray(inp["Wl1"], np.float32),
            bl1=np.asarray(inp["bl1"], np.float32).reshape(-1, 1),
            Wl=np.asarray(inp["Wl"], np.float32),
            bl=np.asarray(inp["bl"], np.float32).reshape(-1, 1),
        )
        for i, (wm, bv) in enumerate(zip(Ws, bs)):
            m[f"W{i+1}"] = W4p if i == 3 else wm.astype(BF16)
            m[f"B{i+1}"] = np.broadcast_to(
                np.asarray(bv, np.float32), (P, len(bv))).copy()
        in_maps.append(m)
    return in_maps


def kernel(x, edge_index, batch, W1, b1, W2, b2, W3, b3, W4, b4,
           Wl1, bl1, Wl, bl):
    from concourse import bass_utils

    x = np.asarray(x)
    prep = _preprocess(x, np.asarray(edge_index), np.asarray(batch))
    bs = [np.asarray(b) for b in (b1, b2, b3, b4)]
    has_bias = tuple(bool(np.any(b != 0)) for b in bs)

    key = (has_bias, tuple(prep["plan"]["cols_rw"].ravel().tolist()))
    if key not in _CACHE:
        _CACHE[key] = build_program(prep["plan"], has_bias)
    nc = _CACHE[key]

    inp = dict(W1=W1, b1=b1, W2=W2, b2=b2, W3=W3, b3=b3, W4=W4, b4=b4,
               Wl1=Wl1, bl1=bl1, Wl=Wl, bl=bl)
    in_maps = _make_in_maps(prep, inp)
    res = bass_utils.run_bass_kernel_spmd(
        nc, in_maps, core_ids=list(range(N_CORES)))
    out = res.results[0]["out_head"]
    return np.ascontiguousarray(out.T.astype(np.float32))
